# revision 6
# baseline (speedup 1.0000x reference)
"""Causal attention head (B=4, S=4096, D=512, E=64) on 8 TRN2 NeuronCores.

Sharding: per batch b, core pair (2b, 2b+1); each core owns 2048 queries
(zig-zag slots) and projects K/V for the full sequence.

v2 structure (vs baseline):
 - Per-512-seq-group input tiles with DMA issued in consumption order and
   pool-rotation backpressure, so projections chase the input DMA stream.
 - K and V^T projections run as col-tiled concurrent matmul pairs
   (wk -> psum[0:64], wv -> psum[64:128]); one DVE copy moves both to SBUF.
 - V^T -> V[keys, e] via 32x32 block-swap SBUF DMAs + DVE StreamTranspose
   (no more 128 LDWEIGHTS-bound V matmuls).
 - K^T/Q^T partition-halves duplicated by SBUF->SBUF DMA (scalar queue)
   instead of engine copies.
 - Attention interleaved with late projection groups in program order so the
   PE never waits for the full projection phase.
 - PE warm-up matmuls + exp table-load primer at t=0 hide HAM ramp and the
   ~2.7us activation table DMA.
 - Flash-style attention unchanged: transposed scores S^T = K_chunk @ Q^T,
   exp on ScalarE, PV with a ones-column appended to V so the softmax
   denominator falls out of the same matmul. Uniform per-slot key-chunk
   counts; zig-zag query assignment; diagonal/zero masks multiply exp output.
All matmul inputs bf16 (pre-cast on host). Output f32.
"""

import sys

sys.path.insert(0, "/opt/trn_rl_repo")

import numpy as np
import ml_dtypes

from concourse import bacc, mybir
from concourse import tile
from concourse.bass_utils import run_bass_kernel_spmd

BF16 = ml_dtypes.bfloat16
F32 = mybir.dt.float32
BF = mybir.dt.bfloat16

B, S, D, E = 4, 4096, 512, 64
P = 128
NQ = 2048          # queries per core
QBLK = 512         # query block
NCH = D // P       # 4 contraction chunks for projections
NG = S // QBLK     # 8 seq groups of 512
NQG = NQ // QBLK   # 4 query groups
QSTARTS = {0: [0, 1024, 2048, 3072], 1: [512, 1536, 2560, 3584]}
SLOT_J = [8, 16, 24, 32]  # uniform per-slot key-chunk counts (all cores)

_CACHE = {}
LAST_RESULT = None


def _build():
    nc = bacc.Bacc(
        "TRN2",
        target_bir_lowering=False,
        debug=False,
        enable_asserts=True,
        num_devices=8,
    )

    xqt_d = nc.declare_dram_parameter("xqt", [D, NQ], BF, isOutput=False)
    xkt_d = nc.declare_dram_parameter("xkt", [D, S], BF, isOutput=False)
    xvt_d = nc.declare_dram_parameter("xvt", [D, S], BF, isOutput=False)
    wq = nc.declare_dram_parameter("wq", [D, E], BF, isOutput=False)  # pre-scaled 1/8
    wk = nc.declare_dram_parameter("wk", [D, E], BF, isOutput=False)
    wv = nc.declare_dram_parameter("wv", [D, E], BF, isOutput=False)
    masks = nc.declare_dram_parameter("masks", [P, 8 * QBLK], BF, isOutput=False)
    ident = nc.declare_dram_parameter("ident", [P, P], F32, isOutput=False)
    zout = nc.declare_dram_parameter("z", [NQ, E], F32, isOutput=True)

    with tile.TileContext(nc) as tc:
        with (
            tc.tile_pool(name="const", bufs=1) as const,
            tc.tile_pool(name="xk", bufs=5) as xkp,
            tc.tile_pool(name="xv", bufs=5) as xvp,
            tc.tile_pool(name="xq", bufs=4) as xqp,
            tc.tile_pool(name="proj", bufs=1) as proj,
            tc.tile_pool(name="work", bufs=4) as work,
            tc.tile_pool(name="epi", bufs=2) as epi,
            tc.tile_pool(name="psP", bufs=2, space="PSUM") as psP,
            tc.tile_pool(name="psS", bufs=2, space="PSUM") as psS,
            tc.tile_pool(name="psZ", bufs=2, space="PSUM") as psZ,
        ):
            # ---- constants (DMA first: tiny) ----
            ident_sb = const.tile([P, P], F32, tag="ident")
            nc.sync.dma_start(out=ident_sb[:, :], in_=ident[:, :])
            wq_sb = const.tile([P, NCH, E], BF, tag="wq")
            wk_sb = const.tile([P, NCH, E], BF, tag="wk")
            wv_sb = const.tile([P, NCH, E], BF, tag="wv")
            for w_dram, w_sb in ((wk, wk_sb), (wv, wv_sb), (wq, wq_sb)):
                nc.sync.dma_start(
                    out=w_sb[:, :, :],
                    in_=w_dram.rearrange("(c p) e -> p c e", p=P),
                )

            # ---- input tiles (per 512-seq group) ----
            xk_t = [
                xkp.tile([P, NCH, QBLK], BF, tag="xk", name=f"xk_t{g}")
                for g in range(NG)
            ]
            xv_t = [
                xvp.tile([P, NCH, QBLK], BF, tag="xv", name=f"xv_t{g}")
                for g in range(NG)
            ]
            xq_t = [
                xqp.tile([P, NCH, QBLK], BF, tag="xq", name=f"xq_t{b}")
                for b in range(NQG)
            ]
            masks_sb = const.tile([P, 8 * QBLK], BF, tag="masks")

            def load_seg(dram, t, g):
                nc.sync.dma_start(
                    out=t[:, :, :],
                    in_=dram[:, g * QBLK : (g + 1) * QBLK].rearrange(
                        "(c p) r -> p c r", p=P
                    ),
                )

            # DMA issue order == desired arrival order (sync HW queue).
            load_seg(xkt_d, xk_t[0], 0)
            load_seg(xvt_d, xv_t[0], 0)
            load_seg(xqt_d, xq_t[0], 0)
            load_seg(xkt_d, xk_t[1], 1)
            load_seg(xvt_d, xv_t[1], 1)
            nc.sync.dma_start(out=masks_sb[:, :], in_=masks[:, :])
            load_seg(xqt_d, xq_t[1], 1)
            for g in range(2, NG):
                load_seg(xkt_d, xk_t[g], g)
                load_seg(xvt_d, xv_t[g], g)
                if g < NQG:
                    load_seg(xqt_d, xq_t[g], g)

            # ---- projected-data tiles ----
            kv_g = [
                proj.tile([P, QBLK], BF, tag=f"kv{g}", name=f"kv_g{g}")
                for g in range(NG)
            ]
            ktb_g = [
                proj.tile([P, QBLK], BF, tag=f"kt{g}", name=f"ktb_g{g}")
                for g in range(NG)
            ]
            qt_b = [
                proj.tile([P, QBLK], BF, tag=f"qt{b}", name=f"qt_b{b}")
                for b in range(NQG)
            ]
            vp_g = [
                proj.tile([P, NCH, E + 1], BF, tag=f"vp{g}", name=f"vp_g{g}")
                for g in range(NG)
            ]
            for g in range(NG):
                nc.gpsimd.memset(vp_g[g][:, :, E : E + 1], 1.0)

            # ---- PE warm-up (HAM ramp) + exp table-load primer ----
            warm_ps = psS.tile([P, 2 * QBLK], F32, tag="st")
            for _ in range(16):
                nc.tensor.matmul(
                    warm_ps[0:E, 0:256],
                    lhsT=wk_sb[:, 0, :],
                    rhs=wk_sb[:, :, :].rearrange("p c e -> p (c e)")[:, 0:256],
                    start=True,
                    stop=True,
                    skip_group_check=True,
                )
            primer = const.tile([1, 8], BF, tag="primer")
            nc.scalar.activation(
                out=primer,
                in_=ident_sb[0:1, 0:8],
                func=mybir.ActivationFunctionType.Exp,
            )

            # ---- projection of one 512-seq group ----
            def proj_group(g):
                kvps = psP.tile([P, QBLK], F32, tag="kv")
                for c in range(NCH):
                    nc.tensor.matmul(
                        kvps[0:E, :],
                        lhsT=wk_sb[:, c, :],
                        rhs=xk_t[g][:, c, :],
                        start=(c == 0),
                        stop=(c == NCH - 1),
                        tile_position=(0, 0),
                        skip_group_check=True,
                    )
                    nc.tensor.matmul(
                        kvps[E:P, :],
                        lhsT=wv_sb[:, c, :],
                        rhs=xv_t[g][:, c, :],
                        start=(c == 0),
                        stop=(c == NCH - 1),
                        tile_position=(0, E),
                        skip_group_check=True,
                    )
                # one copy moves K^T (rows 0:64) and V^T (rows 64:128) to SBUF
                nc.vector.tensor_copy(kv_g[g], kvps)
                # duplicate K^T into partitions 64:128 for the row-tiled scores
                nc.scalar.dma_start(
                    out=ktb_g[g][E:P, :], in_=kv_g[g][0:E, :]
                )
                # V^T -> V[keys, e]: block-swap DMAs + 32x32 StreamTranspose
                vpre = work.tile([P, NCH, E], BF, tag="vpre")
                kv3 = kv_g[g].rearrange("p (c k) -> p c k", c=NCH)
                for bi in range(2):
                    for bj in range(4):
                        nc.scalar.dma_start(
                            out=vpre[
                                32 * bj : 32 * bj + 32, :, 32 * bi : 32 * bi + 32
                            ],
                            in_=kv3[
                                E + 32 * bi : E + 32 * bi + 32,
                                :,
                                32 * bj : 32 * bj + 32,
                            ],
                        )
                for bj in range(4):
                    nc.vector.transpose(
                        vp_g[g][32 * bj : 32 * bj + 32, :, 0:E],
                        vpre[32 * bj : 32 * bj + 32, :, :],
                    )
                if g < NQG:
                    qps = psP.tile([P, QBLK], F32, tag="kv")
                    for c in range(NCH):
                        nc.tensor.matmul(
                            qps[0:E, :],
                            lhsT=wq_sb[:, c, :],
                            rhs=xq_t[g][:, c, :],
                            start=(c == 0),
                            stop=(c == NCH - 1),
                            tile_position=(0, 0),
                            skip_group_check=True,
                        )
                    nc.vector.tensor_copy(qt_b[g][0:E, :], qps[0:E, :])
                    nc.scalar.dma_start(
                        out=qt_b[g][E:P, :], in_=qt_b[g][0:E, :]
                    )

            # ---- attention ----
            attn_state = {}

            def attn_begin(ib):
                attn_state[ib] = {
                    "zps": psZ.tile(
                        [E + 1, QBLK], F32, tag="zt", name=f"zps{ib}"
                    ),
                    "prev": None,
                }

            def emit_pv(ib, pt, jp):
                jmax = SLOT_J[ib]
                zps = attn_state[ib]["zps"]
                for h in range(2):
                    j = 2 * jp + h
                    nc.tensor.matmul(
                        zps,
                        lhsT=vp_g[j // NCH][:, j % NCH, :],
                        rhs=pt[:, h * QBLK : (h + 1) * QBLK],
                        start=(j == 0),
                        stop=(j == jmax - 1),
                        skip_group_check=True,
                    )

            def attn_pairs(ib, jps):
                jmax = SLOT_J[ib]
                st = attn_state[ib]
                for jp in jps:
                    j0, j1 = 2 * jp, 2 * jp + 1
                    sps = psS.tile([P, 2 * QBLK], F32, tag="st")
                    nc.tensor.matmul(
                        sps[:, 0:QBLK],
                        lhsT=kv_g[j0 // NCH][0:E, (j0 % NCH) * P : (j0 % NCH + 1) * P],
                        rhs=qt_b[ib][0:E, :],
                        start=True,
                        stop=True,
                        tile_position=(0, 0),
                        skip_group_check=True,
                    )
                    nc.tensor.matmul(
                        sps[:, QBLK : 2 * QBLK],
                        lhsT=ktb_g[j1 // NCH][E:P, (j1 % NCH) * P : (j1 % NCH + 1) * P],
                        rhs=qt_b[ib][E:P, :],
                        start=True,
                        stop=True,
                        tile_position=(E, 0),
                        skip_group_check=True,
                    )
                    pt = work.tile([P, 2 * QBLK], BF, tag="pt")
                    nc.scalar.activation(
                        out=pt, in_=sps, func=mybir.ActivationFunctionType.Exp
                    )
                    if j0 >= jmax - 8:
                        m = j0 - (jmax - 8)
                        nc.vector.tensor_mul(
                            pt, pt, masks_sb[:, m * QBLK : (m + 2) * QBLK]
                        )
                    if st["prev"] is not None:
                        emit_pv(ib, *st["prev"])
                    st["prev"] = (pt, jp)

            def attn_end(ib):
                st = attn_state[ib]
                emit_pv(ib, *st["prev"])
                zps = st["zps"]
                zsb = epi.tile([E + 1, QBLK], F32, tag="zsb")
                nc.vector.tensor_copy(zsb, zps)
                for u in range(QBLK // P):
                    zbp = psS.tile([P, 2 * QBLK], F32, tag="st")
                    nc.tensor.transpose(
                        zbp[:, 0 : E + 1],
                        zsb[:, u * P : (u + 1) * P],
                        ident_sb[0 : E + 1, 0 : E + 1],
                    )
                    rc = epi.tile([P, 1], F32, tag="rc")
                    nc.vector.reciprocal(rc, zbp[:, E : E + 1])
                    zf = epi.tile([P, E], F32, tag="zf")
                    nc.vector.tensor_scalar_mul(zf, zbp[:, 0:E], rc)
                    row0 = ib * QBLK + u * P
                    nc.sync.dma_start(out=zout[row0 : row0 + P, :], in_=zf)

            # ---- interleaved schedule (program order == PE issue order) ----
            proj_group(0)
            proj_group(1)
            attn_begin(0)
            attn_pairs(0, range(0, 4))      # block 8: chunks 0-7 (groups 0-1)
            attn_end(0)
            attn_begin(1)
            attn_pairs(1, range(0, 4))      # block 16: chunks 0-7
            proj_group(2)
            attn_pairs(1, range(4, 6))      # chunks 8-11 (group 2)
            proj_group(3)
            attn_pairs(1, range(6, 8))      # chunks 12-15 (group 3)
            attn_end(1)
            attn_begin(2)
            attn_pairs(2, range(0, 8))      # block 24: chunks 0-15
            proj_group(4)
            attn_pairs(2, range(8, 10))     # chunks 16-19 (group 4)
            proj_group(5)
            attn_pairs(2, range(10, 12))    # chunks 20-23 (group 5)
            attn_end(2)
            attn_begin(3)
            attn_pairs(3, range(0, 12))     # block 32: chunks 0-23
            proj_group(6)
            attn_pairs(3, range(12, 14))    # chunks 24-27 (group 6)
            proj_group(7)
            attn_pairs(3, range(14, 16))    # chunks 28-31 (group 7)
            attn_end(3)

    nc.compile()
    return nc


def _get_nc():
    if "nc" not in _CACHE:
        _CACHE["nc"] = _build()
    return _CACHE["nc"]


def _ensure_ntff_hook():
    """Install antenv.axon_hooks + NTFF profile hook if the image lacks it."""
    import types

    try:
        from antenv import axon_hooks  # noqa: F401

        return
    except ImportError:
        pass
    import antenv
    from concourse import bass_utils as _bu

    mod = types.ModuleType("antenv.axon_hooks")
    _state = {}
    mod.set_axon_ntff_profile_hook = lambda h: _state.__setitem__("h", h)
    mod.get_axon_ntff_profile_hook = lambda: _state.get("h")
    sys.modules["antenv.axon_hooks"] = mod
    antenv.axon_hooks = mod
    sys.path.insert(0, "/root/.axon_site/trn_agent_boot")
    from trn_boot import _ntff_profile_via_ctypes

    mod.set_axon_ntff_profile_hook(
        _ntff_profile_via_ctypes("/opt/axon/libaxon_pjrt.so")
    )
    _bu.upload_artifacts = lambda tmpdir: f"local://{tmpdir}"


def _make_masks(h):
    kl = np.arange(P)[:, None]
    ql = np.arange(QBLK)[None, :]
    diag = [(kl <= ql - P * t).astype(np.float32) for t in range(4)]
    ones = np.ones((P, QBLK), np.float32)
    zero = np.zeros((P, QBLK), np.float32)
    tiles = diag + [zero] * 4 if h == 0 else [ones] * 4 + diag
    return np.concatenate(tiles, axis=1).astype(BF16)


def kernel(key_inputs, value_inputs, query_inputs, Wq, Wk, Wv):
    global LAST_RESULT
    import os

    key_inputs = np.asarray(key_inputs, dtype=np.float32)
    value_inputs = np.asarray(value_inputs, dtype=np.float32)
    query_inputs = np.asarray(query_inputs, dtype=np.float32)
    wq_b = (np.asarray(Wq, dtype=np.float32) * 0.125).astype(BF16)
    wk_b = np.asarray(Wk, dtype=np.float32).astype(BF16)
    wv_b = np.asarray(Wv, dtype=np.float32).astype(BF16)
    masks_np = [_make_masks(0), _make_masks(1)]
    ident_np = np.eye(P, dtype=np.float32)

    in_maps = []
    for c in range(8):
        b, h = c // 2, c % 2
        xq_c = np.concatenate(
            [query_inputs[b, q0 : q0 + QBLK] for q0 in QSTARTS[h]], axis=0
        )
        xk_c = key_inputs[b]
        xv_c = value_inputs[b]
        in_maps.append(
            {
                "xqt": np.ascontiguousarray(xq_c.T).astype(BF16),
                "xkt": np.ascontiguousarray(xk_c.T).astype(BF16),
                "xvt": np.ascontiguousarray(xv_c.T).astype(BF16),
                "wq": wq_b,
                "wk": wk_b,
                "wv": wv_b,
                "masks": masks_np[h],
                "ident": ident_np,
            }
        )

    nc = _get_nc()
    trace = bool(int(os.environ.get("KERNEL_TRACE", "0")))
    if trace:
        _ensure_ntff_hook()
    res = run_bass_kernel_spmd(
        nc,
        in_maps,
        core_ids=list(range(8)),
        trace=trace,
        tmpdir=os.environ.get("KERNEL_TRACE_DIR") or None,
    )
    LAST_RESULT = res

    out = np.empty((B, S, E), dtype=np.float32)
    for c in range(8):
        b, h = c // 2, c % 2
        z = np.asarray(res.results[c]["z"], dtype=np.float32)
        for ib, q0 in enumerate(QSTARTS[h]):
            out[b, q0 : q0 + QBLK] = z[ib * QBLK : (ib + 1) * QBLK]
    return out
